# revision 27
# baseline (speedup 1.0000x reference)
"""NT-Xent loss on 8 TRN2 NeuronCores.

Reference computes, for z = concat(z1, z2) (2N=8192 rows, D=256):
    zn  = z / max(||z||, eps)
    sim = (zn @ zn.T) / T, diag masked to -1e9
    loss = mean_i( logsumexp_j sim[i, j] - sim[i, pos(i)] ),  pos(i) = (i + N) % 2N

Sharding: 2N rows split into 8 blocks of 1024. Core c receives zn.T with
columns rotated left by c*1024, so its own rows sit at columns 0:1024 on
every core (compile-time constant access patterns, identical program on
all cores).

Per core: 64 tiles of [128 rows x 1024 cols] flowing through 4 PSUM slots
(4 slots of [128,1024]f32 = all 8 banks) so the TensorE producer runs
ahead of the exp consumers instead of serializing with them.  TensorE
computes each tile with fp8(e4m3) DoubleRow matmuls (K=256 as 2 stacked
K=128 planes; the ISA caps the moving tensor at 512 columns/matmul).
The exp+rowsum of each tile runs on one of TWO engines in parallel:
  - ScalarE (ACT): true exp via activation(Exp, scale, accum_out)
  - VectorE (DVE): custom 8-stage op  X = (alpha*p + beta)^2 + gamma;
    out = X^8, accum_out = sum(X^8).  X is the deg-2 Taylor of
    exp(logit/8), so X^8 ~ exp(logit) (loss rel-err ~1e-4)
The self-similarity diagonal is killed before exp by accumulating
-784 (= logit -43.75) onto it with a small eye-matmul into PSUM, so exp
underflows to ~0 (those tiles always go to ACT; the DVE approximation
is only valid on the off-diagonal logit range).  The positive logits
are computed on the HOST from the same quantized fp8 rows (8192
length-256 dot products, exact).  Host: loss = mean(log(S) - pos).
"""

import math
import sys

if "/opt/trn_rl_repo" not in sys.path:
    sys.path.insert(0, "/opt/trn_rl_repo")

from operator import add as _operator_add

import ml_dtypes
import numpy as np

import concourse.bass as bass
import concourse.mybir as mybir
import concourse.tile as tile
from concourse import bacc
from concourse.bass_utils import run_bass_kernel_spmd

N = 4096
D = 256
TWO_N = 2 * N          # 8192
TEMPERATURE = 0.07
EPS = 1e-8
N_CORES = 8
ROWS_PER_CORE = TWO_N // N_CORES   # 1024
M_TILES = ROWS_PER_CORE // 128     # 8 row-tiles of 128
CB = 1024                          # psum tile / column-block width
N_CB = TWO_N // CB                 # 8 column blocks per row-tile
DMA_CB = 2048                      # DMA chunk width (4 chunks)

FP8_SCALE = 16.0                   # zn elements ~N(0,1/256) -> ~N(0,1)
LAM = 1.0 / (256.0 * TEMPERATURE)  # psum (=256*cos) -> logit
MU = LAM / 8.0                     # DVE computes exp(logit/8)^8
ALPHA = MU / math.sqrt(2.0)
BETA = math.sqrt(0.5)
GAMMA = 0.5
EYE_A = 28.0                       # diag pre-add: 28*-28 = -784 -> logit -43.75
EYE_B = -28.0

# engine per (m, cb): 'A' = ScalarE true exp, 'V' = DVE X^8 approx.
# cb 0 (self-diag) must be 'A'.  33 A / 31 V balances measured rates.
ASSIGN = {}
for _m in range(M_TILES):
    for _cb in range(N_CB):
        ASSIGN[(_m, _cb)] = "A" if _cb % 2 == 0 else "V"

_cached = {}


def _register_exp8_op():
    """Register the custom DVE op NTX_EXP8_REDUCE in concourse.dve_ops.OPS
    (the documented extension point; sha computed here so the pin check
    passes).  Idempotent."""
    import concourse.dve_ops as dve_ops
    from concourse.dve_spec import Spec, Src0, C0, C1, C2, Zero, sq, lower
    from concourse.dve_spec import _has_src1
    from concourse.dve_uop import DveOpSpec

    name = "NTX_EXP8_REDUCE"
    for op in dve_ops.OPS:
        if op.name == name:
            return op

    def _ref(in0, in1, s0, s1, imm2):
        x = (in0.astype(np.float32) * np.float32(s0) + np.float32(s1)) ** 2 + np.float32(imm2)
        b = ((x ** 2) ** 2) ** 2
        b = b.astype(np.float32)
        return b, b.reshape(b.shape[0], -1).sum(axis=-1, keepdims=True)

    body = sq(sq(sq(sq(Src0 * C0 + C1) + C2)))
    spec = Spec(body=body, accum=_operator_add, accum_init=Zero, reference=_ref)

    row = dve_ops._CUSTOM_DVE_ROW_BASE + len(dve_ops.OPS)
    shas = {}
    for ver in ("v3", "v4"):
        s = DveOpSpec(name=name, opcode=row, uops=lower(spec, ver=ver),
                      rd1_en=_has_src1(spec))
        shas[ver] = s.sha(ver)
    op = dve_ops.DveOp(name, spec, subdim=False, uops_sha=shas)
    dve_ops.OPS.append(op)
    dve_ops._SUB_OPCODE_FOR_NAME[name] = row
    return op


def _build_bass(m_tiles=M_TILES):
    f32 = mybir.dt.float32
    bf16 = mybir.dt.bfloat16
    fp8 = mybir.dt.float8e4
    exp8_op = _register_exp8_op()
    nc = bacc.Bacc("TRN2", target_bir_lowering=False, debug=False)

    # cst layout: [:, 0:128] = +28*I (stationary eye), [:, 128:2176] = four
    # 512-col blocks with a -28 diagonal at offset off*128 (off = m%4)
    znt = nc.declare_dram_parameter("znt", [D, TWO_N], fp8, isOutput=False)
    cst = nc.declare_dram_parameter("cst", [128, 128 + 4 * 512], fp8, isOutput=False)
    acc_out = nc.declare_dram_parameter(
        "acc", [128, 2 * m_tiles * N_CB], f32, isOutput=True
    )

    n_tiles = m_tiles * N_CB
    with tile.TileContext(nc) as tc:
        with (
            tc.tile_pool(name="zchunks", bufs=1) as zpool,
            tc.tile_pool(name="consts", bufs=1) as cpool,
            tc.tile_pool(name="stats", bufs=1) as spool,
            tc.tile_pool(name="exout", bufs=2) as expool,
            tc.tile_pool(name="psum", bufs=4, space=bass.MemorySpace.PSUM) as ppool,
        ):
            cst_t = cpool.tile([128, 128 + 4 * 512], fp8, tag="cst")
            nc.gpsimd.dma_start(cst_t[:], cst[:])
            eye_l = cst_t[:, 0:128]

            # znt chunks as [128, 2, DMA_CB]: plane k = contraction rows
            # k*128:(k+1)*128 (DoubleRow matmul layout).  Plane 0 via sync
            # queue, plane 1 via gpsimd queue (parallel descriptor issue).
            zt = []
            for ch in range(TWO_N // DMA_CB):
                t = zpool.tile([128, 2, DMA_CB], fp8, tag=f"z_{ch}")
                zt.append(t)
            warm = cpool.tile([128, 1], f32, tag="warm")
            for ch in range(TWO_N // DMA_CB):
                if ch == 0:
                    # split the first chunk so the first tiles' matmuls can
                    # start as soon as the leading half lands
                    nc.sync.dma_start(zt[0][:, 0, 0:CB], znt[0:128, 0:CB])
                    nc.gpsimd.dma_start(zt[0][:, 1, 0:CB], znt[128:256, 0:CB])
                    nc.sync.dma_start(zt[0][:, 0, CB:DMA_CB], znt[0:128, CB:DMA_CB])
                    nc.gpsimd.dma_start(
                        zt[0][:, 1, CB:DMA_CB], znt[128:256, CB:DMA_CB]
                    )
                    # ACT Exp-table preload behind the critical first DMAs
                    nc.gpsimd.memset(warm[:], 0.0)
                    nc.scalar.activation(
                        out=warm[:], in_=warm[:],
                        func=mybir.ActivationFunctionType.Exp, bias=0.0, scale=1.0,
                    )
                    continue
                nc.sync.dma_start(
                    zt[ch][:, 0, :], znt[0:128, ch * DMA_CB : (ch + 1) * DMA_CB]
                )
                nc.gpsimd.dma_start(
                    zt[ch][:, 1, :], znt[128:256, ch * DMA_CB : (ch + 1) * DMA_CB]
                )

            acc_a_t = spool.tile([128, n_tiles], f32, tag="acc_a")
            acc_v_t = spool.tile([128, n_tiles], f32, tag="acc_v")

            dr = mybir.MatmulPerfMode.DoubleRow
            for m in range(m_tiles):
                moff = m * 128
                for cb in range(N_CB):
                    ps = ppool.tile([128, CB], f32, tag="ps")
                    lhsT = zt[0][:, :, moff : moff + 128]
                    zch = zt[cb // 2]
                    coff = (cb % 2) * CB
                    nn_d = m // 4 if cb == 0 else -1
                    for nn in range(CB // 512):
                        dst = ps[:, nn * 512 : (nn + 1) * 512]
                        rhs = zch[:, :, coff + nn * 512 : coff + (nn + 1) * 512]
                        if nn == nn_d:
                            # pre-add -784 on the self-sim diagonal so
                            # exp() underflows to 0
                            off = m % 4
                            nc.tensor.matmul(
                                dst,
                                lhsT=eye_l,
                                rhs=cst_t[:, 128 + off * 512 : 128 + (off + 1) * 512],
                                start=True,
                                stop=False,
                            )
                            nc.tensor.matmul(
                                dst, lhsT=lhsT, rhs=rhs,
                                start=False, stop=True, perf_mode=dr,
                            )
                        else:
                            nc.tensor.matmul(
                                dst, lhsT=lhsT, rhs=rhs,
                                start=True, stop=True, perf_mode=dr,
                            )
                    col = m * N_CB + cb
                    if ASSIGN[(m, cb)] == "A":
                        ex = expool.tile([128, CB], bf16, tag="aex")
                        nc.scalar.activation(
                            out=ex[:],
                            in_=ps[:],
                            func=mybir.ActivationFunctionType.Exp,
                            bias=0.0,
                            scale=LAM,
                            accum_out=acc_a_t[:, col : col + 1],
                        )
                    else:
                        ex = expool.tile([128, CB], bf16, tag="vex")
                        nc.vector._custom_dve(
                            exp8_op,
                            out=ex[:],
                            in0=ps[:],
                            s0=ALPHA,
                            s1=BETA,
                            imm2=GAMMA,
                            accum_out=acc_v_t[:, col : col + 1],
                        )
                if m == m_tiles // 2 - 1:
                    # ship the first half of the accumulators early so the
                    # final output DMA only carries the second half
                    h = n_tiles // 2
                    nc.sync.dma_start(acc_out[:, 0:h], acc_a_t[:, 0:h])
                    nc.gpsimd.dma_start(
                        acc_out[:, n_tiles : n_tiles + h], acc_v_t[:, 0:h]
                    )

            h = n_tiles // 2
            nc.sync.dma_start(acc_out[:, h:n_tiles], acc_a_t[:, h:])
            nc.gpsimd.dma_start(acc_out[:, n_tiles + h :], acc_v_t[:, h:])

    nc.compile()
    return nc


def _prepare_inputs(z1, z2):
    z = np.concatenate([np.asarray(z1), np.asarray(z2)], axis=0).astype(np.float32)
    norms = np.maximum(np.sqrt((z.astype(np.float64) ** 2).sum(-1)), EPS)
    zn = (z / norms[:, None]).astype(np.float32)
    znq = (FP8_SCALE * zn).astype(ml_dtypes.float8_e4m3fn)
    znt = np.ascontiguousarray(znq.T)  # [D, 2N]
    cst = np.zeros((128, 128 + 4 * 512), dtype=np.float32)
    ll = np.arange(128)
    cst[ll, ll] = EYE_A
    for off in range(4):
        cst[ll, 128 + off * 512 + off * 128 + ll] = EYE_B
    cst = cst.astype(ml_dtypes.float8_e4m3fn)
    in_maps = []
    for c in range(N_CORES):
        znt_c = np.ascontiguousarray(np.roll(znt, -c * ROWS_PER_CORE, axis=1))
        in_maps.append({"znt": znt_c, "cst": cst})
    # positive logits computed host-side from the same quantized rows
    zq = znq.astype(np.float32)
    pd = (zq[:N] * zq[N:]).sum(-1) * LAM          # [N]
    pos_logit = np.concatenate([pd, pd]).astype(np.float64)
    return in_maps, pos_logit


def kernel(z1, z2):
    if "nc" not in _cached:
        _cached["nc"] = _build_bass()
    nc = _cached["nc"]
    in_maps, pos_logit = _prepare_inputs(z1, z2)
    res = run_bass_kernel_spmd(nc, in_maps, core_ids=list(range(N_CORES)))
    results = res.results

    a_cols = np.array(
        [ASSIGN[(m, cb)] == "A" for m in range(M_TILES) for cb in range(N_CB)]
    )
    n_tiles = M_TILES * N_CB
    per_row_loss = np.zeros(TWO_N, dtype=np.float64)
    for c in range(N_CORES):
        # acc [128, 8*M]: element [l, m*8+cb] sums tile (m, cb) of rows
        # c*1024 + m*128 + l; diag already excluded on-device.
        acc_all = np.asarray(results[c]["acc"], dtype=np.float64)
        acc_a, acc_v = acc_all[:, :n_tiles], acc_all[:, n_tiles:]
        acc = np.where(a_cols[None, :], acc_a, acc_v)
        S = acc.reshape(128, M_TILES, N_CB).sum(-1)  # [128, M]
        rows = np.log(S.T.reshape(-1))
        per_row_loss[c * ROWS_PER_CORE : (c + 1) * ROWS_PER_CORE] = rows
    per_row_loss -= pos_logit
    return np.float32(per_row_loss.mean())


# revision 35
# speedup vs baseline: 1.0087x; 1.0087x over previous
"""NT-Xent loss on 8 TRN2 NeuronCores.

Reference computes, for z = concat(z1, z2) (2N=8192 rows, D=256):
    zn  = z / max(||z||, eps)
    sim = (zn @ zn.T) / T, diag masked to -1e9
    loss = mean_i( logsumexp_j sim[i, j] - sim[i, pos(i)] ),  pos(i) = (i + N) % 2N

Sharding: 2N rows split into 8 blocks of 1024. Core c receives zn.T with
columns rotated left by c*1024, so its own rows sit at columns 0:1024 on
every core (compile-time constant access patterns, identical program on
all cores).

Per core: 64 tiles of [128 rows x 1024 cols] flowing through 4 PSUM slots
(4 slots of [128,1024]f32 = all 8 banks) so the TensorE producer runs
ahead of the exp consumers instead of serializing with them.  TensorE
computes each tile with fp8(e4m3) DoubleRow matmuls (K=256 as 2 stacked
K=128 planes; the ISA caps the moving tensor at 512 columns/matmul).
The exp+rowsum of each tile runs on one of TWO engines in parallel:
  - ScalarE (ACT): true exp via activation(Exp, scale, accum_out)
  - VectorE (DVE): custom 8-stage op  X = (alpha*p + beta)^2 + gamma;
    out = X^8, accum_out = sum(X^8).  X is the deg-2 Taylor of
    exp(logit/8), so X^8 ~ exp(logit) (loss rel-err ~1e-4)
The self-similarity diagonal is killed before exp by accumulating
-784 (= logit -43.75) onto it with a small eye-matmul into PSUM, so exp
underflows to ~0 (those tiles always go to ACT; the DVE approximation
is only valid on the off-diagonal logit range).  The positive logits
are computed on the HOST from the same quantized fp8 rows (8192
length-256 dot products, exact).  Host: loss = mean(log(S) - pos).
"""

import math
import sys

if "/opt/trn_rl_repo" not in sys.path:
    sys.path.insert(0, "/opt/trn_rl_repo")

from operator import add as _operator_add

import ml_dtypes
import numpy as np

import concourse.bass as bass
import concourse.mybir as mybir
import concourse.tile as tile
from concourse import bacc
from concourse.bass_utils import run_bass_kernel_spmd

N = 4096
D = 256
TWO_N = 2 * N          # 8192
TEMPERATURE = 0.07
EPS = 1e-8
N_CORES = 8
ROWS_PER_CORE = TWO_N // N_CORES   # 1024
M_TILES = ROWS_PER_CORE // 128     # 8 row-tiles of 128
CB = 1024                          # psum tile / column-block width
N_CB = TWO_N // CB                 # 8 column blocks per row-tile
DMA_CB = 2048                      # DMA chunk width (4 chunks)

FP8_SCALE = 16.0                   # zn elements ~N(0,1/256) -> ~N(0,1)
LAM = 1.0 / (256.0 * TEMPERATURE)  # psum (=256*cos) -> logit
MU = LAM / 8.0                     # DVE computes exp(logit/8)^8
ALPHA = MU / math.sqrt(2.0)
BETA = math.sqrt(0.5)
GAMMA = 0.5
EYE_A = 28.0                       # diag pre-add: 28*-28 = -784 -> logit -43.75
EYE_B = -28.0

# engine per (m, cb): 'A' = ScalarE true exp, 'V' = DVE X^8 approx.
# cb 0 (self-diag) must be 'A'.  33 A / 31 V balances measured rates.
ASSIGN = {}
for _m in range(M_TILES):
    for _cb in range(N_CB):
        ASSIGN[(_m, _cb)] = "A" if _cb % 2 == 0 else "V"

_cached = {}


def _register_exp8_op():
    """Register the custom DVE op NTX_EXP8_REDUCE in concourse.dve_ops.OPS
    (the documented extension point; sha computed here so the pin check
    passes).  Idempotent."""
    import concourse.dve_ops as dve_ops
    from concourse.dve_spec import Spec, Src0, C0, C1, C2, Zero, sq, lower
    from concourse.dve_spec import _has_src1
    from concourse.dve_uop import DveOpSpec

    name = "NTX_EXP8_REDUCE"
    for op in dve_ops.OPS:
        if op.name == name:
            return op

    def _ref(in0, in1, s0, s1, imm2):
        x = (in0.astype(np.float32) * np.float32(s0) + np.float32(s1)) ** 2 + np.float32(imm2)
        b = ((x ** 2) ** 2) ** 2
        b = b.astype(np.float32)
        return b, b.reshape(b.shape[0], -1).sum(axis=-1, keepdims=True)

    body = sq(sq(sq(sq(Src0 * C0 + C1) + C2)))
    spec = Spec(body=body, accum=_operator_add, accum_init=Zero, reference=_ref)

    row = dve_ops._CUSTOM_DVE_ROW_BASE + len(dve_ops.OPS)
    shas = {}
    for ver in ("v3", "v4"):
        s = DveOpSpec(name=name, opcode=row, uops=lower(spec, ver=ver),
                      rd1_en=_has_src1(spec))
        shas[ver] = s.sha(ver)
    op = dve_ops.DveOp(name, spec, subdim=False, uops_sha=shas)
    dve_ops.OPS.append(op)
    dve_ops._SUB_OPCODE_FOR_NAME[name] = row
    return op


def _build_bass(m_tiles=M_TILES):
    f32 = mybir.dt.float32
    bf16 = mybir.dt.bfloat16
    fp8 = mybir.dt.float8e4
    exp8_op = _register_exp8_op()
    nc = bacc.Bacc("TRN2", target_bir_lowering=False, debug=False)

    # cst layout: [:, 0:128] = +28*I (stationary eye), [:, 128:2176] = four
    # 512-col blocks with a -28 diagonal at offset off*128 (off = m%4)
    znt = nc.declare_dram_parameter("znt", [D, TWO_N], fp8, isOutput=False)
    cst = nc.declare_dram_parameter("cst", [128, 128 + 4 * 512], fp8, isOutput=False)
    acc_out = nc.declare_dram_parameter(
        "acc", [128, 2 * m_tiles * N_CB], f32, isOutput=True
    )

    n_tiles = m_tiles * N_CB
    with tile.TileContext(nc) as tc:
        with (
            tc.tile_pool(name="zchunks", bufs=1) as zpool,
            tc.tile_pool(name="consts", bufs=1) as cpool,
            tc.tile_pool(name="stats", bufs=1) as spool,
            tc.tile_pool(name="exout", bufs=2) as expool,
            tc.tile_pool(name="psum", bufs=4, space=bass.MemorySpace.PSUM) as ppool,
        ):
            cst_t = cpool.tile([128, 128 + 4 * 512], fp8, tag="cst")
            nc.gpsimd.dma_start(cst_t[:], cst[:])
            eye_l = cst_t[:, 0:128]

            # znt chunks as [128, 2, DMA_CB]: plane k = contraction rows
            # k*128:(k+1)*128 (DoubleRow matmul layout).  Plane 0 via sync
            # queue, plane 1 via gpsimd queue (parallel descriptor issue).
            zt = []
            for ch in range(TWO_N // DMA_CB):
                t = zpool.tile([128, 2, DMA_CB], fp8, tag=f"z_{ch}")
                zt.append(t)
            warm = cpool.tile([128, 1], f32, tag="warm")
            for ch in range(TWO_N // DMA_CB):
                if ch == 0:
                    # split the first chunk so the first tiles' matmuls can
                    # start as soon as the leading half lands
                    nc.sync.dma_start(zt[0][:, 0, 0:CB], znt[0:128, 0:CB])
                    nc.gpsimd.dma_start(zt[0][:, 1, 0:CB], znt[128:256, 0:CB])
                    nc.sync.dma_start(zt[0][:, 0, CB:DMA_CB], znt[0:128, CB:DMA_CB])
                    nc.gpsimd.dma_start(
                        zt[0][:, 1, CB:DMA_CB], znt[128:256, CB:DMA_CB]
                    )
                    # ACT Exp-table preload behind the critical first DMAs
                    nc.gpsimd.memset(warm[:], 0.0)
                    nc.scalar.activation(
                        out=warm[:], in_=warm[:],
                        func=mybir.ActivationFunctionType.Exp, bias=0.0, scale=1.0,
                    )
                    continue
                nc.sync.dma_start(
                    zt[ch][:, 0, :], znt[0:128, ch * DMA_CB : (ch + 1) * DMA_CB]
                )
                nc.gpsimd.dma_start(
                    zt[ch][:, 1, :], znt[128:256, ch * DMA_CB : (ch + 1) * DMA_CB]
                )

            acc_a_t = spool.tile([128, n_tiles], f32, tag="acc_a")
            acc_v_t = spool.tile([128, n_tiles], f32, tag="acc_v")

            dr = mybir.MatmulPerfMode.DoubleRow
            for m in range(m_tiles):
                moff = m * 128
                for cb in range(N_CB):
                    ps = ppool.tile([128, CB], f32, tag="ps")
                    lhsT = zt[0][:, :, moff : moff + 128]
                    zch = zt[cb // 2]
                    coff = (cb % 2) * CB
                    nn_d = m // 4 if cb == 0 else -1
                    for nn in range(CB // 512):
                        dst = ps[:, nn * 512 : (nn + 1) * 512]
                        rhs = zch[:, :, coff + nn * 512 : coff + (nn + 1) * 512]
                        if nn == nn_d:
                            # pre-add -784 on the self-sim diagonal so
                            # exp() underflows to 0
                            off = m % 4
                            nc.tensor.matmul(
                                dst,
                                lhsT=eye_l,
                                rhs=cst_t[:, 128 + off * 512 : 128 + (off + 1) * 512],
                                start=True,
                                stop=False,
                            )
                            nc.tensor.matmul(
                                dst, lhsT=lhsT, rhs=rhs,
                                start=False, stop=True, perf_mode=dr,
                            )
                        else:
                            nc.tensor.matmul(
                                dst, lhsT=lhsT, rhs=rhs,
                                start=True, stop=True, perf_mode=dr,
                            )
                    col = m * N_CB + cb
                    if ASSIGN[(m, cb)] == "A":
                        ex = expool.tile([128, CB], bf16, tag="aex")
                        nc.scalar.activation(
                            out=ex[:],
                            in_=ps[:],
                            func=mybir.ActivationFunctionType.Exp,
                            bias=0.0,
                            scale=LAM,
                            accum_out=acc_a_t[:, col : col + 1],
                        )
                    else:
                        ex = expool.tile([128, CB], bf16, tag="vex")
                        nc.vector._custom_dve(
                            exp8_op,
                            out=ex[:],
                            in0=ps[:],
                            s0=ALPHA,
                            s1=BETA,
                            imm2=GAMMA,
                            accum_out=acc_v_t[:, col : col + 1],
                        )
                if m == m_tiles // 2 - 1:
                    # ship the first half of the accumulators early so the
                    # final output DMA only carries the second half
                    h = n_tiles // 2
                    nc.sync.dma_start(acc_out[:, 0:h], acc_a_t[:, 0:h])
                    nc.gpsimd.dma_start(
                        acc_out[:, n_tiles : n_tiles + h], acc_v_t[:, 0:h]
                    )

            h = n_tiles // 2
            nc.sync.dma_start(acc_out[:, h:n_tiles], acc_a_t[:, h:])
            nc.gpsimd.dma_start(acc_out[:, n_tiles + h :], acc_v_t[:, h:])

    nc.compile()
    return nc


def _prepare_inputs(z1, z2):
    z = np.concatenate([np.asarray(z1), np.asarray(z2)], axis=0).astype(np.float32)
    norms = np.maximum(np.sqrt((z.astype(np.float64) ** 2).sum(-1)), EPS)
    zn = (z / norms[:, None]).astype(np.float32)
    znq = (FP8_SCALE * zn).astype(ml_dtypes.float8_e4m3fn)
    znt = np.ascontiguousarray(znq.T)  # [D, 2N]
    cst = np.zeros((128, 128 + 4 * 512), dtype=np.float32)
    ll = np.arange(128)
    cst[ll, ll] = EYE_A
    for off in range(4):
        cst[ll, 128 + off * 512 + off * 128 + ll] = EYE_B
    cst = cst.astype(ml_dtypes.float8_e4m3fn)
    in_maps = []
    for c in range(N_CORES):
        znt_c = np.ascontiguousarray(np.roll(znt, -c * ROWS_PER_CORE, axis=1))
        in_maps.append({"znt": znt_c, "cst": cst})
    # positive logits computed host-side from the same quantized rows
    zq = znq.astype(np.float32)
    pd = (zq[:N] * zq[N:]).sum(-1) * LAM          # [N]
    pos_logit = np.concatenate([pd, pd]).astype(np.float64)
    return in_maps, pos_logit


def kernel(z1, z2):
    if "nc" not in _cached:
        _cached["nc"] = _build_bass()
    nc = _cached["nc"]
    in_maps, pos_logit = _prepare_inputs(z1, z2)
    res = run_bass_kernel_spmd(nc, in_maps, core_ids=list(range(N_CORES)))
    results = res.results

    a_cols = np.array(
        [ASSIGN[(m, cb)] == "A" for m in range(M_TILES) for cb in range(N_CB)]
    )
    n_tiles = M_TILES * N_CB
    per_row_loss = np.zeros(TWO_N, dtype=np.float64)
    for c in range(N_CORES):
        # acc [128, 8*M]: element [l, m*8+cb] sums tile (m, cb) of rows
        # c*1024 + m*128 + l; diag already excluded on-device.
        acc_all = np.asarray(results[c]["acc"], dtype=np.float64)
        acc_a, acc_v = acc_all[:, :n_tiles], acc_all[:, n_tiles:]
        acc = np.where(a_cols[None, :], acc_a, acc_v)
        S = acc.reshape(128, M_TILES, N_CB).sum(-1)  # [128, M]
        rows = np.log(S.T.reshape(-1))
        per_row_loss[c * ROWS_PER_CORE : (c + 1) * ROWS_PER_CORE] = rows
    per_row_loss -= pos_logit
    return np.float32(per_row_loss.mean())
